# revision 11
# baseline (speedup 1.0000x reference)
"""Trainium2 Bass kernel for nn_MEPG_Loss (MEPG policy-gradient loss).

Math (forward only; stop_gradient is identity):
    h   = tanh(states[s,:,t] @ W1 + b1)                  [S,T,H]
    mu  = h @ W2 + b2                                    [S,T,A]
    ll[s,t] = -0.5*(||a[s,:,t]-mu||^2/SD + A*log(2*pi*SD))
    base = rewards.T - ALPHA*ll.T ; cum = base with row T-2 += row T-1
    A_hat = cum - log(0.5)
    out = einsum('ts,us->', A_hat, ll.T)/S
        = sum_s (sum_t A_hat[t,s]) * (sum_t ll[t,s]) / S

So only per-simulation reductions are needed:
    q_sum[s]  = sum_{t,d} (mu - a)^2,   q_last[s] = sum_d (mu - a)^2 at t=T-1
    R[s] = sum_t rewards,               r_last[s] = rewards[s,T-1]
    L = -0.5*q_sum/SD + T*C0 ;          ll_last = -0.5*q_last/SD + C0
    A_sum = R + r_last - ALPHA*(L + ll_last) - T*log(0.5)
    out = sum_s A_sum*L / S

Device layout (per core, 256 sims, processed as 64 quads of 4 sims):
    - states quad loaded at SBUF partitions {32j..32j+16}, j = sim-in-quad
    - mm1: 4 row-tiled matmuls (K=16) -> h_pre [128, 512] per sim (4 PSUM banks)
    - tanh on ScalarE over all 4 banks in one op -> h SBUF [128, 2048]
    - mm2: 4 col-tiled matmuls lhsT=W2 -> mu at psum partitions {32j+d}
    - diff: 4 diag-tiled identity matmuls accumulate (b2 - a) onto mu
    - DVE tensor_tensor_reduce: SD = diff^2 (full tile) + per-partition t-sum
    - final K=128 matmul with a 4-block selection matrix sums over d
Final combine (tiny) is done on host in float64.
"""

import os
import sys

import numpy as np

if not any(os.path.isdir(os.path.join(p, "concourse")) for p in sys.path if p):
    sys.path.insert(0, "/opt/trn_rl_repo")

import concourse.bacc as bacc
import concourse.tile as tile
from concourse import mybir
from concourse.bass_utils import run_bass_kernel_spmd

# Problem constants (hardcoded per contract)
S, D, A, T, HID = 2048, 16, 4, 512, 128
N_CORES = 8
SS = S // N_CORES          # 256 sims per core
NQ = SS // 4               # 64 quads per core
SD_VAR = 0.04
ALPHA = 0.1
MAX_POSITION = 1.0

F32 = mybir.dt.float32


def _build_program(stage=8):
    nc = bacc.Bacc("TRN2", target_bir_lowering=False, debug=False)

    states_d = nc.dram_tensor("states", [SS, D, T], F32, kind="ExternalInput").ap()
    aadj_d = nc.dram_tensor("aadj", [SS, A, T], F32, kind="ExternalInput").ap()
    rew_d = nc.dram_tensor("rewards", [SS, T], F32, kind="ExternalInput").ap()
    w1f_d = nc.dram_tensor("w1full", [128, HID], F32, kind="ExternalInput").ap()
    w2_d = nc.dram_tensor("w2", [HID, A], F32, kind="ExternalInput").ap()
    i4_d = nc.dram_tensor("i4rep", [128, 16], F32, kind="ExternalInput").ap()
    sel_d = nc.dram_tensor("sel", [128, A], F32, kind="ExternalInput").ap()
    b1_d = nc.dram_tensor("b1col", [HID, 1], F32, kind="ExternalInput").ap()

    outq_d = nc.dram_tensor("outq", [A, 2 * NQ], F32, kind="ExternalOutput").ap()
    outr_d = nc.dram_tensor("outr", [128, 4], F32, kind="ExternalOutput").ap()

    with tile.TileContext(nc) as tc:
        with (
            tc.tile_pool(name="consts", bufs=1) as consts,
            tc.tile_pool(name="stp", bufs=3) as stp,
            tc.tile_pool(name="atp", bufs=3) as atp,
            tc.tile_pool(name="hsb", bufs=2) as hsb,
            tc.tile_pool(name="sdp", bufs=2) as sdp,
            tc.tile_pool(name="acc", bufs=1) as accp,
            tc.tile_pool(name="outs", bufs=1) as outp,
            tc.tile_pool(name="psh", bufs=1, space="PSUM") as psh,
            tc.tile_pool(name="psm", bufs=1, space="PSUM") as psm,
            tc.tile_pool(name="psq", bufs=1, space="PSUM") as psq,
        ):
            # constants
            w1t = consts.tile([128, HID], F32, tag="w1t")
            w2t = consts.tile([HID, A], F32, tag="w2t")
            i4t = consts.tile([128, 16], F32, tag="i4t")
            selt = consts.tile([128, A], F32, tag="selt")
            b1t = consts.tile([HID, 1], F32, tag="b1t")
            nc.sync.dma_start(out=w1t[:], in_=w1f_d)
            nc.sync.dma_start(out=w2t[:], in_=w2_d)
            nc.sync.dma_start(out=i4t[:], in_=i4_d)
            nc.sync.dma_start(out=selt[:], in_=sel_d)
            nc.sync.dma_start(out=b1t[:], in_=b1_d)

            # per-quad accumulators (written one column per quad)
            accq = accp.tile([128, NQ], F32, tag="accq")
            qlg = accp.tile([128, NQ], F32, tag="qlg")

            # persistent mu psum banks; partitions outside {32j+d, d<4} must be
            # exactly zero (tensor_tensor_reduce reads the full tile), and the
            # matmuls below never write them, so zero once here.
            mu_tiles = [psm.tile([128, T], F32, tag=f"mu{i}", name=f"mu{i}")
                        for i in range(2)]
            for mt in mu_tiles:
                nc.vector.memset(mt[:], 0.0)

            for g in range(NQ):
                st = stp.tile([128, T], F32, tag="st")
                at = atp.tile([128, T], F32, tag="at")
                for j in range(4):
                    nc.sync.dma_start(out=st[32 * j:32 * j + D, :],
                                      in_=states_d[4 * g + j])
                    nc.sync.dma_start(out=at[32 * j:32 * j + A, :],
                                      in_=aadj_d[4 * g + j])

                if stage < 2:
                    continue
                hps = psh.tile([128, 4 * T], F32, tag="hps")
                for j in range(4):
                    nc.tensor.matmul(
                        out=hps[:, T * j:T * (j + 1)],
                        lhsT=w1t[32 * j:32 * j + D, :],
                        rhs=st[32 * j:32 * j + D, :],
                        start=True, stop=True,
                        tile_position=(32 * j, 0),
                    )

                h = hsb.tile([128, 4 * T], F32, tag="h")
                nc.scalar.activation(
                    out=h[:], in_=hps[:],
                    func=mybir.ActivationFunctionType.Tanh,
                    bias=b1t[:], scale=1.0,
                )

                if stage < 3:
                    continue
                mu = mu_tiles[g % 2]
                for j in range(4):
                    nc.tensor.matmul(
                        out=mu[32 * j:32 * j + A, :],
                        lhsT=w2t[:],
                        rhs=h[:, T * j:T * (j + 1)],
                        start=True, stop=False,
                        tile_position=(0, 32 * j),
                        skip_group_check=True,
                    )
                if stage < 4:
                    continue
                for j in range(4):
                    nc.tensor.matmul(
                        out=mu[32 * j:32 * j + A, :],
                        lhsT=i4t[32 * j:32 * j + A, 4 * j:4 * j + A],
                        rhs=at[32 * j:32 * j + A, :],
                        start=False, stop=True,
                        tile_position=(32 * j, 32 * j),
                        skip_group_check=True,
                    )

                if stage < 5:
                    continue
                dfc = sdp.tile([128, T], F32, tag="dfc")
                nc.vector.tensor_copy(dfc[:], mu[:])
                if stage < 6:
                    continue
                sd = sdp.tile([128, T], F32, tag="sd")
                nc.vector.scalar_tensor_tensor(
                    out=sd[:], in0=mu[:], scalar=1.0, in1=dfc[:],
                    op0=mybir.AluOpType.mult, op1=mybir.AluOpType.mult,
                    accum_out=accq[:, g:g + 1],
                )
                if stage < 7:
                    continue
                nc.vector.tensor_copy(qlg[:, g:g + 1], sd[:, T - 1:T])

            # rewards: R and r_last for two blocks of 128 sims
            outr_sb = outp.tile([128, 4], F32, tag="outr")
            for b in range(2):
                rw = stp.tile([128, T], F32, tag="rw")
                nc.sync.dma_start(out=rw[:], in_=rew_d[128 * b:128 * b + 128, :])
                nc.vector.tensor_reduce(
                    out=outr_sb[:, b:b + 1], in_=rw[:],
                    axis=mybir.AxisListType.X, op=mybir.AluOpType.add,
                )
                nc.vector.tensor_copy(outr_sb[:, 2 + b:3 + b], rw[:, T - 1:T])

            # sum over d: QS[j, g] = sum_d accq[32j+d, g]
            outq_sb = outp.tile([A, 2 * NQ], F32, tag="outq")
            if stage >= 8:
                qs_ps = psq.tile([A, NQ], F32, tag="qs")
                ql_ps = psq.tile([A, NQ], F32, tag="ql")
                nc.tensor.matmul(out=qs_ps[:], lhsT=selt[:], rhs=accq[:],
                                 start=True, stop=True)
                nc.tensor.matmul(out=ql_ps[:], lhsT=selt[:], rhs=qlg[:],
                                 start=True, stop=True)
                nc.vector.tensor_copy(outq_sb[:, 0:NQ], qs_ps[:])
                nc.vector.tensor_copy(outq_sb[:, NQ:2 * NQ], ql_ps[:])
            else:
                nc.vector.memset(outq_sb[:], 0.0)
                if stage < 6:
                    nc.vector.memset(accq[:], 0.0)
                if stage < 7:
                    nc.vector.memset(qlg[:], 0.0)

            nc.sync.dma_start(out=outq_d, in_=outq_sb[:])
            nc.sync.dma_start(out=outr_d, in_=outr_sb[:])

    nc.finalize()
    return nc


_NC_CACHE = {}


def _get_program(stage=8):
    key = f"nc{stage}"
    if key not in _NC_CACHE:
        _NC_CACHE[key] = _build_program(stage)
    return _NC_CACHE[key]


def _make_consts(W1, b1, W2, b2):
    w1full = np.zeros((128, HID), dtype=np.float32)
    i4rep = np.zeros((128, 16), dtype=np.float32)
    sel = np.zeros((128, A), dtype=np.float32)
    for j in range(4):
        w1full[32 * j:32 * j + D, :] = W1
        for d in range(A):
            i4rep[32 * j + d, 4 * j + d] = 1.0
            sel[32 * j + d, j] = 1.0
    return {
        "w1full": w1full,
        "w2": np.ascontiguousarray(W2.astype(np.float32)),
        "i4rep": i4rep,
        "sel": sel,
        "b1col": np.ascontiguousarray(b1.astype(np.float32).reshape(HID, 1)),
    }


def kernel(states, actions, rewards, W1, b1, W2, b2, _run_kwargs=None):
    states = np.ascontiguousarray(np.asarray(states, dtype=np.float32))
    actions = np.asarray(actions, dtype=np.float32)
    rewards = np.ascontiguousarray(np.asarray(rewards, dtype=np.float32))
    W1 = np.asarray(W1, dtype=np.float32)
    b1 = np.asarray(b1, dtype=np.float32)
    W2 = np.asarray(W2, dtype=np.float32)
    b2 = np.asarray(b2, dtype=np.float32)

    aadj = np.ascontiguousarray(b2[None, :, None] - actions)
    consts = _make_consts(W1, b1, W2, b2)

    in_maps = []
    for c in range(N_CORES):
        sl = slice(SS * c, SS * (c + 1))
        m = {
            "states": states[sl],
            "aadj": aadj[sl],
            "rewards": rewards[sl],
        }
        m.update(consts)
        in_maps.append(m)

    nc = _get_program()
    res = run_bass_kernel_spmd(nc, in_maps, core_ids=list(range(N_CORES)),
                               **(_run_kwargs or {}))
    results = res.results

    # host combine in float64
    C0 = -0.5 * A * np.log(2.0 * np.pi * SD_VAR)
    mx_pos = np.log(1.0 / (2.0 * MAX_POSITION))
    total = 0.0
    for c in range(N_CORES):
        outq = results[c]["outq"].astype(np.float64)  # [A, 2*NQ]
        outr = results[c]["outr"].astype(np.float64)  # [128, 4]
        qs = outq[:, :NQ].T.reshape(SS)               # s_local = 4g + j
        ql = outq[:, NQ:].T.reshape(SS)
        R = outr[:, 0:2].T.reshape(SS)                # s_local = 128b + p
        rlast = outr[:, 2:4].T.reshape(SS)
        L = -0.5 * qs / SD_VAR + T * C0
        ll_last = -0.5 * ql / SD_VAR + C0
        A_sum = R + rlast - ALPHA * (L + ll_last) - T * mx_pos
        total += np.sum(A_sum * L)
    out = np.float32(total / S)
    if _run_kwargs:
        _NC_CACHE["last_result"] = res
    return out


if __name__ == "__main__":
    rng = np.random.default_rng(0)
    inputs = {
        "states": rng.standard_normal((S, D, T), dtype=np.float32),
        "actions": rng.standard_normal((S, A, T), dtype=np.float32),
        "rewards": rng.standard_normal((S, T), dtype=np.float32),
        "W1": (rng.standard_normal((D, HID)) / np.sqrt(D)).astype(np.float32),
        "b1": np.zeros(HID, np.float32),
        "W2": (rng.standard_normal((HID, A)) / np.sqrt(HID)).astype(np.float32),
        "b2": np.zeros(A, np.float32),
    }
    print("result:", kernel(**inputs))


# revision 12
# speedup vs baseline: 1.4935x; 1.4935x over previous
"""Trainium2 Bass kernel for nn_MEPG_Loss (MEPG policy-gradient loss).

Math (forward only; stop_gradient is identity):
    h   = tanh(states[s,:,t] @ W1 + b1)                  [S,T,H]
    mu  = h @ W2 + b2                                    [S,T,A]
    ll[s,t] = -0.5*(||a[s,:,t]-mu||^2/SD + A*log(2*pi*SD))
    base = rewards.T - ALPHA*ll.T ; cum = base with row T-2 += row T-1
    A_hat = cum - log(0.5)
    out = einsum('ts,us->', A_hat, ll.T)/S
        = sum_s (sum_t A_hat[t,s]) * (sum_t ll[t,s]) / S

So only per-simulation reductions are needed:
    q_sum[s]  = sum_{t,d} (mu - a)^2,   q_last[s] = sum_d (mu - a)^2 at t=T-1
    R[s] = sum_t rewards,               r_last[s] = rewards[s,T-1]
    L = -0.5*q_sum/SD + T*C0 ;          ll_last = -0.5*q_last/SD + C0
    A_sum = R + r_last - ALPHA*(L + ll_last) - T*log(0.5)
    out = sum_s A_sum*L / S

Device layout (per core, 256 sims, processed as 64 quads of 4 sims,
grouped in blocks of 4 quads for DMA batching):
    - states quad loaded (bf16) at SBUF partitions {32j..32j+16}, j = sim-in-quad
    - mm1 (bf16): 4 row-tiled matmuls (K=16) -> h_pre [128, 512] psum fp32
    - tanh on ScalarE over 4 banks in one op -> h SBUF [128, 2048] bf16
    - mm2 (bf16): 4 col-tiled matmuls lhsT=W2 -> mu at psum partitions {32j+d}
    - diff (fp32): 4 diag-tiled identity matmuls accumulate (b2 - a) onto mu
    - DVE: copy diff psum->sbuf, then scalar_tensor_tensor squares it with
      free-axis sum into a per-quad accumulator column
    - final K=128 matmul with a 4-block selection matrix sums over d
Final combine (tiny) is done on host in float64.
"""

import os
import sys

import numpy as np

if not any(os.path.isdir(os.path.join(p, "concourse")) for p in sys.path if p):
    sys.path.insert(0, "/opt/trn_rl_repo")

import ml_dtypes

import concourse.bacc as bacc
import concourse.tile as tile
from concourse import mybir
from concourse.bass_utils import run_bass_kernel_spmd

# Problem constants (hardcoded per contract)
S, D, A, T, HID = 2048, 16, 4, 512, 128
N_CORES = 8
SS = S // N_CORES          # 256 sims per core
NQ = SS // 4               # 64 quads per core
QB = 4                     # quads per DMA block
NB = NQ // QB              # 16 blocks
SD_VAR = 0.04
ALPHA = 0.1
MAX_POSITION = 1.0

F32 = mybir.dt.float32
BF16 = mybir.dt.bfloat16
NP_BF16 = ml_dtypes.bfloat16


def _build_program():
    nc = bacc.Bacc("TRN2", target_bir_lowering=False, debug=False)

    states_d = nc.dram_tensor("states", [SS, D, T], BF16, kind="ExternalInput").ap()
    aadj_d = nc.dram_tensor("aadj", [SS, A, T], F32, kind="ExternalInput").ap()
    rew_d = nc.dram_tensor("rewards", [SS, T], F32, kind="ExternalInput").ap()
    w1f_d = nc.dram_tensor("w1full", [128, HID], BF16, kind="ExternalInput").ap()
    w2_d = nc.dram_tensor("w2", [HID, A], BF16, kind="ExternalInput").ap()
    i4_d = nc.dram_tensor("i4rep", [128, 16], F32, kind="ExternalInput").ap()
    sel_d = nc.dram_tensor("sel", [128, A], F32, kind="ExternalInput").ap()
    b1_d = nc.dram_tensor("b1col", [HID, 1], F32, kind="ExternalInput").ap()

    outq_d = nc.dram_tensor("outq", [A, 2 * NQ], F32, kind="ExternalOutput").ap()
    outr_d = nc.dram_tensor("outr", [128, 4], F32, kind="ExternalOutput").ap()

    with tile.TileContext(nc) as tc:
        with (
            tc.tile_pool(name="consts", bufs=1) as consts,
            tc.tile_pool(name="stp", bufs=2) as stp,
            tc.tile_pool(name="atp", bufs=2) as atp,
            tc.tile_pool(name="hsb", bufs=2) as hsb,
            tc.tile_pool(name="sdp", bufs=2) as sdp,
            tc.tile_pool(name="acc", bufs=1) as accp,
            tc.tile_pool(name="outs", bufs=1) as outp,
            tc.tile_pool(name="psh", bufs=1, space="PSUM") as psh,
            tc.tile_pool(name="psm", bufs=1, space="PSUM") as psm,
            tc.tile_pool(name="psq", bufs=1, space="PSUM") as psq,
        ):
            # constants
            w1t = consts.tile([128, HID], BF16, tag="w1t")
            w2t = consts.tile([HID, A], BF16, tag="w2t")
            i4t = consts.tile([128, 16], F32, tag="i4t")
            selt = consts.tile([128, A], F32, tag="selt")
            b1t = consts.tile([HID, 1], F32, tag="b1t")
            nc.sync.dma_start(out=w1t[:], in_=w1f_d)
            nc.sync.dma_start(out=w2t[:], in_=w2_d)
            nc.sync.dma_start(out=i4t[:], in_=i4_d)
            nc.sync.dma_start(out=selt[:], in_=sel_d)
            nc.sync.dma_start(out=b1t[:], in_=b1_d)

            # per-quad accumulators (written one column per quad)
            accq = accp.tile([128, NQ], F32, tag="accq")
            qlg = accp.tile([128, NQ], F32, tag="qlg")

            # persistent mu psum banks; partitions outside {32j+d, d<4} must be
            # exactly zero (the DVE square reads the full tile), and the
            # matmuls below never write them, so zero once here.
            mu_tiles = [psm.tile([128, T], F32, tag=f"mu{i}", name=f"mu{i}")
                        for i in range(2)]
            for mt in mu_tiles:
                nc.vector.memset(mt[:], 0.0)

            for b in range(NB):
                s0 = 4 * QB * b
                # batched loads: one DMA per sim-slot j covers all QB quads
                st = stp.tile([128, QB * T], BF16, tag="st")
                at = atp.tile([128, QB * T], F32, tag="at")
                for j in range(4):
                    src = states_d[s0 + j:s0 + 4 * QB:4]          # [QB, D, T]
                    nc.sync.dma_start(
                        out=st[32 * j:32 * j + D, :].rearrange(
                            "d (q t) -> d q t", q=QB),
                        in_=src.rearrange("q d t -> d q t"),
                    )
                    asrc = aadj_d[s0 + j:s0 + 4 * QB:4]           # [QB, A, T]
                    nc.sync.dma_start(
                        out=at[32 * j:32 * j + A, :].rearrange(
                            "d (q t) -> d q t", q=QB),
                        in_=asrc.rearrange("q d t -> d q t"),
                    )

                for q in range(QB):
                    g = QB * b + q
                    hps = psh.tile([128, 4 * T], F32, tag="hps")
                    for j in range(4):
                        nc.tensor.matmul(
                            out=hps[:, T * j:T * (j + 1)],
                            lhsT=w1t[32 * j:32 * j + D, :],
                            rhs=st[32 * j:32 * j + D, T * q:T * (q + 1)],
                            start=True, stop=True,
                            tile_position=(32 * j, 0),
                        )

                    h = hsb.tile([128, 4 * T], BF16, tag="h")
                    nc.scalar.activation(
                        out=h[:], in_=hps[:],
                        func=mybir.ActivationFunctionType.Tanh,
                        bias=b1t[:], scale=1.0,
                    )

                    mu = mu_tiles[g % 2]
                    for j in range(4):
                        nc.tensor.matmul(
                            out=mu[32 * j:32 * j + A, :],
                            lhsT=w2t[:],
                            rhs=h[:, T * j:T * (j + 1)],
                            start=True, stop=False,
                            tile_position=(0, 32 * j),
                            skip_group_check=True,
                        )
                    for j in range(4):
                        nc.tensor.matmul(
                            out=mu[32 * j:32 * j + A, :],
                            lhsT=i4t[32 * j:32 * j + A, 4 * j:4 * j + A],
                            rhs=at[32 * j:32 * j + A, T * q:T * (q + 1)],
                            start=False, stop=True,
                            tile_position=(32 * j, 32 * j),
                            skip_group_check=True,
                        )

                    dfc = sdp.tile([128, T], F32, tag="dfc")
                    nc.vector.tensor_copy(dfc[:], mu[:])
                    sd = sdp.tile([128, T], F32, tag="sd")
                    nc.vector.scalar_tensor_tensor(
                        out=sd[:], in0=mu[:], scalar=1.0, in1=dfc[:],
                        op0=mybir.AluOpType.mult, op1=mybir.AluOpType.mult,
                        accum_out=accq[:, g:g + 1],
                    )
                    nc.vector.tensor_copy(qlg[:, g:g + 1], sd[:, T - 1:T])

            # rewards: R and r_last for two blocks of 128 sims
            outr_sb = outp.tile([128, 4], F32, tag="outr")
            for rb in range(2):
                rw = stp.tile([128, T], F32, tag="rw")
                nc.sync.dma_start(out=rw[:], in_=rew_d[128 * rb:128 * rb + 128, :])
                nc.vector.tensor_reduce(
                    out=outr_sb[:, rb:rb + 1], in_=rw[:],
                    axis=mybir.AxisListType.X, op=mybir.AluOpType.add,
                )
                nc.vector.tensor_copy(outr_sb[:, 2 + rb:3 + rb], rw[:, T - 1:T])

            # sum over d: QS[j, g] = sum_d accq[32j+d, g]
            outq_sb = outp.tile([A, 2 * NQ], F32, tag="outq")
            qs_ps = psq.tile([A, NQ], F32, tag="qs")
            ql_ps = psq.tile([A, NQ], F32, tag="ql")
            nc.tensor.matmul(out=qs_ps[:], lhsT=selt[:], rhs=accq[:],
                             start=True, stop=True)
            nc.tensor.matmul(out=ql_ps[:], lhsT=selt[:], rhs=qlg[:],
                             start=True, stop=True)
            nc.vector.tensor_copy(outq_sb[:, 0:NQ], qs_ps[:])
            nc.vector.tensor_copy(outq_sb[:, NQ:2 * NQ], ql_ps[:])

            nc.sync.dma_start(out=outq_d, in_=outq_sb[:])
            nc.sync.dma_start(out=outr_d, in_=outr_sb[:])

    nc.finalize()
    return nc


_NC_CACHE = {}


def _get_program():
    if "nc" not in _NC_CACHE:
        _NC_CACHE["nc"] = _build_program()
    return _NC_CACHE["nc"]


def _make_consts(W1, b1, W2, b2):
    w1full = np.zeros((128, HID), dtype=NP_BF16)
    i4rep = np.zeros((128, 16), dtype=np.float32)
    sel = np.zeros((128, A), dtype=np.float32)
    for j in range(4):
        w1full[32 * j:32 * j + D, :] = W1.astype(NP_BF16)
        for d in range(A):
            i4rep[32 * j + d, 4 * j + d] = 1.0
            sel[32 * j + d, j] = 1.0
    return {
        "w1full": w1full,
        "w2": np.ascontiguousarray(W2.astype(NP_BF16)),
        "i4rep": i4rep,
        "sel": sel,
        "b1col": np.ascontiguousarray(b1.astype(np.float32).reshape(HID, 1)),
    }


def kernel(states, actions, rewards, W1, b1, W2, b2, _run_kwargs=None):
    states = np.ascontiguousarray(np.asarray(states, dtype=np.float32)
                                  .astype(NP_BF16))
    actions = np.asarray(actions, dtype=np.float32)
    rewards = np.ascontiguousarray(np.asarray(rewards, dtype=np.float32))
    W1 = np.asarray(W1, dtype=np.float32)
    b1 = np.asarray(b1, dtype=np.float32)
    W2 = np.asarray(W2, dtype=np.float32)
    b2 = np.asarray(b2, dtype=np.float32)

    aadj = np.ascontiguousarray(b2[None, :, None] - actions)
    consts = _make_consts(W1, b1, W2, b2)

    in_maps = []
    for c in range(N_CORES):
        sl = slice(SS * c, SS * (c + 1))
        m = {
            "states": states[sl],
            "aadj": aadj[sl],
            "rewards": rewards[sl],
        }
        m.update(consts)
        in_maps.append(m)

    nc = _get_program()
    res = run_bass_kernel_spmd(nc, in_maps, core_ids=list(range(N_CORES)),
                               **(_run_kwargs or {}))
    results = res.results

    # host combine in float64
    C0 = -0.5 * A * np.log(2.0 * np.pi * SD_VAR)
    mx_pos = np.log(1.0 / (2.0 * MAX_POSITION))
    total = 0.0
    for c in range(N_CORES):
        outq = results[c]["outq"].astype(np.float64)  # [A, 2*NQ]
        outr = results[c]["outr"].astype(np.float64)  # [128, 4]
        qs = outq[:, :NQ].T.reshape(SS)               # s_local = 4g + j
        ql = outq[:, NQ:].T.reshape(SS)
        R = outr[:, 0:2].T.reshape(SS)                # s_local = 128b + p
        rlast = outr[:, 2:4].T.reshape(SS)
        L = -0.5 * qs / SD_VAR + T * C0
        ll_last = -0.5 * ql / SD_VAR + C0
        A_sum = R + rlast - ALPHA * (L + ll_last) - T * mx_pos
        total += np.sum(A_sum * L)
    out = np.float32(total / S)
    if _run_kwargs:
        _NC_CACHE["last_result"] = res
    return out


if __name__ == "__main__":
    rng = np.random.default_rng(0)
    inputs = {
        "states": rng.standard_normal((S, D, T), dtype=np.float32),
        "actions": rng.standard_normal((S, A, T), dtype=np.float32),
        "rewards": rng.standard_normal((S, T), dtype=np.float32),
        "W1": (rng.standard_normal((D, HID)) / np.sqrt(D)).astype(np.float32),
        "b1": np.zeros(HID, np.float32),
        "W2": (rng.standard_normal((HID, A)) / np.sqrt(HID)).astype(np.float32),
        "b2": np.zeros(A, np.float32),
    }
    print("result:", kernel(**inputs))


# revision 14
# speedup vs baseline: 2.1705x; 1.4533x over previous
"""Trainium2 Bass kernel for nn_MEPG_Loss (MEPG policy-gradient loss).

Math (forward only; stop_gradient is identity):
    h   = tanh(states[s,:,t] @ W1 + b1)                  [S,T,H]
    mu  = h @ W2 + b2                                    [S,T,A]
    ll[s,t] = -0.5*(||a[s,:,t]-mu||^2/SD + A*log(2*pi*SD))
    base = rewards.T - ALPHA*ll.T ; cum = base with row T-2 += row T-1
    A_hat = cum - log(0.5)
    out = einsum('ts,us->', A_hat, ll.T)/S
        = sum_s (sum_t A_hat[t,s]) * (sum_t ll[t,s]) / S

So only per-simulation reductions are needed:
    q_sum[s]  = sum_{t,d} (mu - a)^2,   q_last[s] = sum_d (mu - a)^2 at t=T-1
    R[s] = sum_t rewards,               r_last[s] = rewards[s,T-1]
    L = -0.5*q_sum/SD + T*C0 ;          ll_last = -0.5*q_last/SD + C0
    A_sum = R + r_last - ALPHA*(L + ll_last) - T*log(0.5)
    out = sum_s A_sum*L / S

Device layout (per core, 256 sims, processed as 64 quads of 4 sims,
grouped in blocks of 4 quads for DMA batching):
    - states quad loaded (bf16) at SBUF partitions {32j..32j+16}, j = sim-in-quad
    - mm1 (bf16): 4 row-tiled matmuls (K=16) -> h_pre [128, 512] psum fp32
    - tanh on ScalarE over 4 banks in one op -> h SBUF [128, 2048] bf16
    - mm2 (bf16): 4 col-tiled matmuls lhsT=W2 -> mu at psum partitions {32j+d}
    - diff (fp32): 4 diag-tiled identity matmuls accumulate (b2 - a) onto mu
    - DVE: copy diff psum->sbuf, then scalar_tensor_tensor squares it with
      free-axis sum into a per-quad accumulator column
    - final K=128 matmul with a 4-block selection matrix sums over d
Final combine (tiny) is done on host in float64.
"""

import os
import sys

import numpy as np

if not any(os.path.isdir(os.path.join(p, "concourse")) for p in sys.path if p):
    sys.path.insert(0, "/opt/trn_rl_repo")

import ml_dtypes

import concourse.bacc as bacc
import concourse.tile as tile
from concourse import mybir
from concourse.bass_utils import run_bass_kernel_spmd

# Problem constants (hardcoded per contract)
S, D, A, T, HID = 2048, 16, 4, 512, 128
N_CORES = 8
SS = S // N_CORES          # 256 sims per core
NQ = SS // 4               # 64 quads per core
QB = 8                     # quads per DMA block
NB = NQ // QB              # 16 blocks
SD_VAR = 0.04
ALPHA = 0.1
MAX_POSITION = 1.0

F32 = mybir.dt.float32
BF16 = mybir.dt.bfloat16
NP_BF16 = ml_dtypes.bfloat16


def _build_program():
    nc = bacc.Bacc("TRN2", target_bir_lowering=False, debug=False)

    states_d = nc.dram_tensor("states", [SS, D, T], BF16, kind="ExternalInput").ap()
    aadj_d = nc.dram_tensor("aadj", [SS, A, T], F32, kind="ExternalInput").ap()
    rew_d = nc.dram_tensor("rewards", [SS, T], F32, kind="ExternalInput").ap()
    w1f_d = nc.dram_tensor("w1full", [128, HID], BF16, kind="ExternalInput").ap()
    w2_d = nc.dram_tensor("w2", [HID, A], BF16, kind="ExternalInput").ap()
    sel_d = nc.dram_tensor("sel", [128, A], F32, kind="ExternalInput").ap()
    b1_d = nc.dram_tensor("b1col", [HID, 1], F32, kind="ExternalInput").ap()

    outq_d = nc.dram_tensor("outq", [A, 2 * NQ], F32, kind="ExternalOutput").ap()
    outr_d = nc.dram_tensor("outr", [128, 4], F32, kind="ExternalOutput").ap()

    with tile.TileContext(nc) as tc:
        with (
            tc.tile_pool(name="consts", bufs=1) as consts,
            tc.tile_pool(name="stp", bufs=2) as stp,
            tc.tile_pool(name="atp", bufs=1) as atp,
            tc.tile_pool(name="hsb", bufs=2) as hsb,
            tc.tile_pool(name="sdp", bufs=2) as sdp,
            tc.tile_pool(name="acc", bufs=1) as accp,
            tc.tile_pool(name="outs", bufs=1) as outp,
            tc.tile_pool(name="psh", bufs=1, space="PSUM") as psh,
            tc.tile_pool(name="psm", bufs=1, space="PSUM") as psm,
            tc.tile_pool(name="psq", bufs=1, space="PSUM") as psq,
        ):
            # constants
            w1t = consts.tile([128, HID], BF16, tag="w1t")
            w2t = consts.tile([HID, A], BF16, tag="w2t")
            selt = consts.tile([128, A], F32, tag="selt")
            b1t = consts.tile([HID, 1], F32, tag="b1t")
            nc.sync.dma_start(out=w1t[:], in_=w1f_d)
            nc.sync.dma_start(out=w2t[:], in_=w2_d)
            nc.sync.dma_start(out=selt[:], in_=sel_d)
            nc.sync.dma_start(out=b1t[:], in_=b1_d)

            # per-quad accumulators (written one column per quad)
            accq = accp.tile([128, NQ], F32, tag="accq")
            qlg = accp.tile([128, NQ], F32, tag="qlg")

            # persistent mu psum banks; partitions outside {32j+d, d<4} must be
            # exactly zero (the DVE square reads the full tile), and the
            # matmuls below never write them, so zero once here.
            mu_tiles = [psm.tile([128, T], F32, tag=f"mu{i}", name=f"mu{i}")
                        for i in range(2)]
            for mt in mu_tiles:
                nc.vector.memset(mt[:], 0.0)
            at_tiles = [atp.tile([128, QB * T], F32, tag=f"at{i}", name=f"at{i}")
                        for i in range(2)]
            for att in at_tiles:
                nc.vector.memset(att[:], 0.0)

            for b in range(NB):
                s0 = 4 * QB * b
                # batched loads: one DMA per sim-slot j covers all QB quads
                st = stp.tile([128, QB * T], BF16, tag="st")
                at = at_tiles[b % 2]
                for j in range(4):
                    src = states_d[s0 + j:s0 + 4 * QB:4]          # [QB, D, T]
                    nc.sync.dma_start(
                        out=st[32 * j:32 * j + D, :].rearrange(
                            "d (q t) -> d q t", q=QB),
                        in_=src.rearrange("q d t -> d q t"),
                    )
                    asrc = aadj_d[s0 + j:s0 + 4 * QB:4]           # [QB, A, T]
                    nc.sync.dma_start(
                        out=at[32 * j:32 * j + A, :].rearrange(
                            "d (q t) -> d q t", q=QB),
                        in_=asrc.rearrange("q d t -> d q t"),
                    )

                for q in range(QB):
                    g = QB * b + q
                    hps = psh.tile([128, 4 * T], F32, tag="hps")
                    for j in range(4):
                        nc.tensor.matmul(
                            out=hps[:, T * j:T * (j + 1)],
                            lhsT=w1t[32 * j:32 * j + D, :],
                            rhs=st[32 * j:32 * j + D, T * q:T * (q + 1)],
                            start=True, stop=True,
                            tile_position=(32 * j, 0),
                        )

                    h = hsb.tile([128, 4 * T], BF16, tag="h")
                    nc.scalar.activation(
                        out=h[:], in_=hps[:],
                        func=mybir.ActivationFunctionType.Tanh,
                        bias=b1t[:], scale=1.0,
                    )

                    mu = mu_tiles[g % 2]
                    for j in range(4):
                        nc.tensor.matmul(
                            out=mu[32 * j:32 * j + A, :],
                            lhsT=w2t[:],
                            rhs=h[:, T * j:T * (j + 1)],
                            start=True, stop=True,
                            tile_position=(0, 32 * j),
                            skip_group_check=True,
                        )

                    # diff = mu + (b2 - a)
                    dfc = sdp.tile([128, T], F32, tag="dfc")
                    nc.vector.tensor_tensor(
                        out=dfc[:], in0=at[:, T * q:T * (q + 1)], in1=mu[:],
                        op=mybir.AluOpType.add,
                    )
                    sd = sdp.tile([128, T], F32, tag="sd")
                    nc.vector.scalar_tensor_tensor(
                        out=sd[:], in0=dfc[:], scalar=1.0, in1=dfc[:],
                        op0=mybir.AluOpType.mult, op1=mybir.AluOpType.mult,
                        accum_out=accq[:, g:g + 1],
                    )
                    nc.vector.tensor_copy(qlg[:, g:g + 1], sd[:, T - 1:T])

            # rewards: R and r_last for two blocks of 128 sims
            outr_sb = outp.tile([128, 4], F32, tag="outr")
            for rb in range(2):
                rw = stp.tile([128, T], F32, tag="rw")
                nc.sync.dma_start(out=rw[:], in_=rew_d[128 * rb:128 * rb + 128, :])
                nc.vector.tensor_reduce(
                    out=outr_sb[:, rb:rb + 1], in_=rw[:],
                    axis=mybir.AxisListType.X, op=mybir.AluOpType.add,
                )
                nc.vector.tensor_copy(outr_sb[:, 2 + rb:3 + rb], rw[:, T - 1:T])

            # sum over d: QS[j, g] = sum_d accq[32j+d, g]
            outq_sb = outp.tile([A, 2 * NQ], F32, tag="outq")
            qs_ps = psq.tile([A, NQ], F32, tag="qs")
            ql_ps = psq.tile([A, NQ], F32, tag="ql")
            nc.tensor.matmul(out=qs_ps[:], lhsT=selt[:], rhs=accq[:],
                             start=True, stop=True)
            nc.tensor.matmul(out=ql_ps[:], lhsT=selt[:], rhs=qlg[:],
                             start=True, stop=True)
            nc.vector.tensor_copy(outq_sb[:, 0:NQ], qs_ps[:])
            nc.vector.tensor_copy(outq_sb[:, NQ:2 * NQ], ql_ps[:])

            nc.sync.dma_start(out=outq_d, in_=outq_sb[:])
            nc.sync.dma_start(out=outr_d, in_=outr_sb[:])

    nc.finalize()
    return nc


_NC_CACHE = {}


def _get_program():
    if "nc" not in _NC_CACHE:
        _NC_CACHE["nc"] = _build_program()
    return _NC_CACHE["nc"]


def _make_consts(W1, b1, W2, b2):
    w1full = np.zeros((128, HID), dtype=NP_BF16)
    sel = np.zeros((128, A), dtype=np.float32)
    for j in range(4):
        w1full[32 * j:32 * j + D, :] = W1.astype(NP_BF16)
        for d in range(A):
            sel[32 * j + d, j] = 1.0
    return {
        "w1full": w1full,
        "w2": np.ascontiguousarray(W2.astype(NP_BF16)),
        "sel": sel,
        "b1col": np.ascontiguousarray(b1.astype(np.float32).reshape(HID, 1)),
    }


def kernel(states, actions, rewards, W1, b1, W2, b2, _run_kwargs=None):
    states = np.ascontiguousarray(np.asarray(states, dtype=np.float32)
                                  .astype(NP_BF16))
    actions = np.asarray(actions, dtype=np.float32)
    rewards = np.ascontiguousarray(np.asarray(rewards, dtype=np.float32))
    W1 = np.asarray(W1, dtype=np.float32)
    b1 = np.asarray(b1, dtype=np.float32)
    W2 = np.asarray(W2, dtype=np.float32)
    b2 = np.asarray(b2, dtype=np.float32)

    aadj = np.ascontiguousarray(b2[None, :, None] - actions)
    consts = _make_consts(W1, b1, W2, b2)

    in_maps = []
    for c in range(N_CORES):
        sl = slice(SS * c, SS * (c + 1))
        m = {
            "states": states[sl],
            "aadj": aadj[sl],
            "rewards": rewards[sl],
        }
        m.update(consts)
        in_maps.append(m)

    nc = _get_program()
    res = run_bass_kernel_spmd(nc, in_maps, core_ids=list(range(N_CORES)),
                               **(_run_kwargs or {}))
    results = res.results

    # host combine in float64
    C0 = -0.5 * A * np.log(2.0 * np.pi * SD_VAR)
    mx_pos = np.log(1.0 / (2.0 * MAX_POSITION))
    total = 0.0
    for c in range(N_CORES):
        outq = results[c]["outq"].astype(np.float64)  # [A, 2*NQ]
        outr = results[c]["outr"].astype(np.float64)  # [128, 4]
        qs = outq[:, :NQ].T.reshape(SS)               # s_local = 4g + j
        ql = outq[:, NQ:].T.reshape(SS)
        R = outr[:, 0:2].T.reshape(SS)                # s_local = 128b + p
        rlast = outr[:, 2:4].T.reshape(SS)
        L = -0.5 * qs / SD_VAR + T * C0
        ll_last = -0.5 * ql / SD_VAR + C0
        A_sum = R + rlast - ALPHA * (L + ll_last) - T * mx_pos
        total += np.sum(A_sum * L)
    out = np.float32(total / S)
    if _run_kwargs:
        _NC_CACHE["last_result"] = res
    return out


if __name__ == "__main__":
    rng = np.random.default_rng(0)
    inputs = {
        "states": rng.standard_normal((S, D, T), dtype=np.float32),
        "actions": rng.standard_normal((S, A, T), dtype=np.float32),
        "rewards": rng.standard_normal((S, T), dtype=np.float32),
        "W1": (rng.standard_normal((D, HID)) / np.sqrt(D)).astype(np.float32),
        "b1": np.zeros(HID, np.float32),
        "W2": (rng.standard_normal((HID, A)) / np.sqrt(HID)).astype(np.float32),
        "b2": np.zeros(A, np.float32),
    }
    print("result:", kernel(**inputs))


# revision 15
# speedup vs baseline: 2.2761x; 1.0486x over previous
"""Trainium2 Bass kernel for nn_MEPG_Loss (MEPG policy-gradient loss).

Math (forward only; stop_gradient is identity):
    h   = tanh(states[s,:,t] @ W1 + b1)                  [S,T,H]
    mu  = h @ W2 + b2                                    [S,T,A]
    ll[s,t] = -0.5*(||a[s,:,t]-mu||^2/SD + A*log(2*pi*SD))
    base = rewards.T - ALPHA*ll.T ; cum = base with row T-2 += row T-1
    A_hat = cum - log(0.5)
    out = einsum('ts,us->', A_hat, ll.T)/S
        = sum_s (sum_t A_hat[t,s]) * (sum_t ll[t,s]) / S

So only per-simulation reductions are needed:
    q_sum[s]  = sum_{t,d} (mu - a)^2,   q_last[s] = sum_d (mu - a)^2 at t=T-1
    R[s] = sum_t rewards,               r_last[s] = rewards[s,T-1]
    L = -0.5*q_sum/SD + T*C0 ;          ll_last = -0.5*q_last/SD + C0
    A_sum = R + r_last - ALPHA*(L + ll_last) - T*log(0.5)
    out = sum_s A_sum*L / S

Device layout (per core, 256 sims, processed as 64 quads of 4 sims,
grouped in blocks of 4 quads for DMA batching):
    - states quad loaded (bf16) at SBUF partitions {32j..32j+16}, j = sim-in-quad
    - mm1 (bf16): 4 row-tiled matmuls (K=16) -> h_pre [128, 512] psum fp32
    - tanh on ScalarE over 4 banks in one op -> h SBUF [128, 2048] bf16
    - mm2 (bf16): 4 col-tiled matmuls lhsT=W2 -> mu at psum partitions {32j+d}
    - diff (fp32): 4 diag-tiled identity matmuls accumulate (b2 - a) onto mu
    - DVE: copy diff psum->sbuf, then scalar_tensor_tensor squares it with
      free-axis sum into a per-quad accumulator column
    - final K=128 matmul with a 4-block selection matrix sums over d
Final combine (tiny) is done on host in float64.
"""

import os
import sys

import numpy as np

if not any(os.path.isdir(os.path.join(p, "concourse")) for p in sys.path if p):
    sys.path.insert(0, "/opt/trn_rl_repo")

import ml_dtypes

import concourse.bacc as bacc
import concourse.tile as tile
from concourse import mybir
from concourse.bass_utils import run_bass_kernel_spmd

# Problem constants (hardcoded per contract)
S, D, A, T, HID = 2048, 16, 4, 512, 128
N_CORES = 8
SS = S // N_CORES          # 256 sims per core
NQ = SS // 4               # 64 quads per core
QB = 8                     # quads per DMA block
NB = NQ // QB              # 16 blocks
SD_VAR = 0.04
ALPHA = 0.1
MAX_POSITION = 1.0

F32 = mybir.dt.float32
BF16 = mybir.dt.bfloat16
NP_BF16 = ml_dtypes.bfloat16


def _build_program():
    nc = bacc.Bacc("TRN2", target_bir_lowering=False, debug=False)

    states_d = nc.dram_tensor("states", [SS, D, T], BF16, kind="ExternalInput").ap()
    aadj_d = nc.dram_tensor("aadj", [SS, A, T], F32, kind="ExternalInput").ap()
    rew_d = nc.dram_tensor("rewards", [SS, T], F32, kind="ExternalInput").ap()
    w1f_d = nc.dram_tensor("w1full", [128, HID], BF16, kind="ExternalInput").ap()
    w2_d = nc.dram_tensor("w2", [HID, A], BF16, kind="ExternalInput").ap()
    sel_d = nc.dram_tensor("sel", [128, A], F32, kind="ExternalInput").ap()
    b1_d = nc.dram_tensor("b1col", [HID, 1], F32, kind="ExternalInput").ap()

    outq_d = nc.dram_tensor("outq", [A, 2 * NQ], F32, kind="ExternalOutput").ap()
    outr_d = nc.dram_tensor("outr", [128, 4], F32, kind="ExternalOutput").ap()

    with tile.TileContext(nc) as tc:
        with (
            tc.tile_pool(name="consts", bufs=1) as consts,
            tc.tile_pool(name="stp", bufs=2) as stp,
            tc.tile_pool(name="atp", bufs=1) as atp,
            tc.tile_pool(name="hsb", bufs=2) as hsb,
            tc.tile_pool(name="sdp", bufs=2) as sdp,
            tc.tile_pool(name="acc", bufs=1) as accp,
            tc.tile_pool(name="outs", bufs=1) as outp,
            tc.tile_pool(name="psh", bufs=1, space="PSUM") as psh,
            tc.tile_pool(name="psm", bufs=1, space="PSUM") as psm,
            tc.tile_pool(name="psq", bufs=1, space="PSUM") as psq,
        ):
            # constants
            w1t = consts.tile([128, HID], BF16, tag="w1t")
            w2t = consts.tile([HID, A], BF16, tag="w2t")
            selt = consts.tile([128, A], F32, tag="selt")
            b1t = consts.tile([HID, 1], F32, tag="b1t")
            nc.sync.dma_start(out=w1t[:], in_=w1f_d)
            nc.sync.dma_start(out=w2t[:], in_=w2_d)
            nc.sync.dma_start(out=selt[:], in_=sel_d)
            nc.sync.dma_start(out=b1t[:], in_=b1_d)

            # per-quad accumulators (written one column per quad)
            accq = accp.tile([128, NQ], F32, tag="accq")
            qlg = accp.tile([128, NQ], F32, tag="qlg")

            # persistent mu psum banks; partitions outside {32j+d, d<4} must be
            # exactly zero (the DVE square reads the full tile), and the
            # matmuls below never write them, so zero once here.
            mu_tiles = [psm.tile([128, T], F32, tag=f"mu{i}", name=f"mu{i}")
                        for i in range(2)]
            for mt in mu_tiles:
                nc.vector.memset(mt[:], 0.0)
            at_tiles = [atp.tile([128, QB * T], F32, tag=f"at{i}", name=f"at{i}")
                        for i in range(2)]
            for att in at_tiles:
                nc.vector.memset(att[:], 0.0)

            for b in range(NB):
                s0 = 4 * QB * b
                # batched loads: one DMA per sim-slot j covers all QB quads
                st = stp.tile([128, QB * T], BF16, tag="st")
                at = at_tiles[b % 2]
                for j in range(4):
                    src = states_d[s0 + j:s0 + 4 * QB:4]          # [QB, D, T]
                    nc.sync.dma_start(
                        out=st[32 * j:32 * j + D, :].rearrange(
                            "d (q t) -> d q t", q=QB),
                        in_=src.rearrange("q d t -> d q t"),
                    )
                    asrc = aadj_d[s0 + j:s0 + 4 * QB:4]           # [QB, A, T]
                    nc.sync.dma_start(
                        out=at[32 * j:32 * j + A, :].rearrange(
                            "d (q t) -> d q t", q=QB),
                        in_=asrc.rearrange("q d t -> d q t"),
                    )

                for q in range(QB):
                    g = QB * b + q
                    # two 2-bank h_pre tiles so mm1 of the next quad can run
                    # while tanh of this quad is still reading the other half
                    hpsA = psh.tile([128, 2 * T], F32, tag="hpsA")
                    hpsB = psh.tile([128, 2 * T], F32, tag="hpsB")
                    for j in range(4):
                        dst = hpsA if j < 2 else hpsB
                        nc.tensor.matmul(
                            out=dst[:, T * (j % 2):T * (j % 2 + 1)],
                            lhsT=w1t[32 * j:32 * j + D, :],
                            rhs=st[32 * j:32 * j + D, T * q:T * (q + 1)],
                            start=True, stop=True,
                            tile_position=(32 * j, 0),
                        )

                    h = hsb.tile([128, 4 * T], BF16, tag="h")
                    nc.scalar.activation(
                        out=h[:, 0:2 * T], in_=hpsA[:],
                        func=mybir.ActivationFunctionType.Tanh,
                        bias=b1t[:], scale=1.0,
                    )
                    nc.scalar.activation(
                        out=h[:, 2 * T:4 * T], in_=hpsB[:],
                        func=mybir.ActivationFunctionType.Tanh,
                        bias=b1t[:], scale=1.0,
                    )

                    mu = mu_tiles[g % 2]
                    for j in range(4):
                        nc.tensor.matmul(
                            out=mu[32 * j:32 * j + A, :],
                            lhsT=w2t[:],
                            rhs=h[:, T * j:T * (j + 1)],
                            start=True, stop=True,
                            tile_position=(0, 32 * j),
                            skip_group_check=True,
                        )

                    # diff = mu + (b2 - a)
                    dfc = sdp.tile([128, T], F32, tag="dfc")
                    nc.vector.tensor_tensor(
                        out=dfc[:], in0=at[:, T * q:T * (q + 1)], in1=mu[:],
                        op=mybir.AluOpType.add,
                    )
                    sd = sdp.tile([128, T], F32, tag="sd")
                    nc.vector.scalar_tensor_tensor(
                        out=sd[:], in0=dfc[:], scalar=1.0, in1=dfc[:],
                        op0=mybir.AluOpType.mult, op1=mybir.AluOpType.mult,
                        accum_out=accq[:, g:g + 1],
                    )
                    nc.vector.tensor_copy(qlg[:, g:g + 1], sd[:, T - 1:T])

            # rewards: R and r_last for two blocks of 128 sims
            outr_sb = outp.tile([128, 4], F32, tag="outr")
            for rb in range(2):
                rw = stp.tile([128, T], F32, tag="rw")
                nc.sync.dma_start(out=rw[:], in_=rew_d[128 * rb:128 * rb + 128, :])
                nc.vector.tensor_reduce(
                    out=outr_sb[:, rb:rb + 1], in_=rw[:],
                    axis=mybir.AxisListType.X, op=mybir.AluOpType.add,
                )
                nc.vector.tensor_copy(outr_sb[:, 2 + rb:3 + rb], rw[:, T - 1:T])

            # sum over d: QS[j, g] = sum_d accq[32j+d, g]
            outq_sb = outp.tile([A, 2 * NQ], F32, tag="outq")
            qs_ps = psq.tile([A, NQ], F32, tag="qs")
            ql_ps = psq.tile([A, NQ], F32, tag="ql")
            nc.tensor.matmul(out=qs_ps[:], lhsT=selt[:], rhs=accq[:],
                             start=True, stop=True)
            nc.tensor.matmul(out=ql_ps[:], lhsT=selt[:], rhs=qlg[:],
                             start=True, stop=True)
            nc.vector.tensor_copy(outq_sb[:, 0:NQ], qs_ps[:])
            nc.vector.tensor_copy(outq_sb[:, NQ:2 * NQ], ql_ps[:])

            nc.sync.dma_start(out=outq_d, in_=outq_sb[:])
            nc.sync.dma_start(out=outr_d, in_=outr_sb[:])

    nc.finalize()
    return nc


_NC_CACHE = {}


def _get_program():
    if "nc" not in _NC_CACHE:
        _NC_CACHE["nc"] = _build_program()
    return _NC_CACHE["nc"]


def _make_consts(W1, b1, W2, b2):
    w1full = np.zeros((128, HID), dtype=NP_BF16)
    sel = np.zeros((128, A), dtype=np.float32)
    for j in range(4):
        w1full[32 * j:32 * j + D, :] = W1.astype(NP_BF16)
        for d in range(A):
            sel[32 * j + d, j] = 1.0
    return {
        "w1full": w1full,
        "w2": np.ascontiguousarray(W2.astype(NP_BF16)),
        "sel": sel,
        "b1col": np.ascontiguousarray(b1.astype(np.float32).reshape(HID, 1)),
    }


def kernel(states, actions, rewards, W1, b1, W2, b2, _run_kwargs=None):
    states = np.ascontiguousarray(np.asarray(states, dtype=np.float32)
                                  .astype(NP_BF16))
    actions = np.asarray(actions, dtype=np.float32)
    rewards = np.ascontiguousarray(np.asarray(rewards, dtype=np.float32))
    W1 = np.asarray(W1, dtype=np.float32)
    b1 = np.asarray(b1, dtype=np.float32)
    W2 = np.asarray(W2, dtype=np.float32)
    b2 = np.asarray(b2, dtype=np.float32)

    aadj = np.ascontiguousarray(b2[None, :, None] - actions)
    consts = _make_consts(W1, b1, W2, b2)

    in_maps = []
    for c in range(N_CORES):
        sl = slice(SS * c, SS * (c + 1))
        m = {
            "states": states[sl],
            "aadj": aadj[sl],
            "rewards": rewards[sl],
        }
        m.update(consts)
        in_maps.append(m)

    nc = _get_program()
    res = run_bass_kernel_spmd(nc, in_maps, core_ids=list(range(N_CORES)),
                               **(_run_kwargs or {}))
    results = res.results

    # host combine in float64
    C0 = -0.5 * A * np.log(2.0 * np.pi * SD_VAR)
    mx_pos = np.log(1.0 / (2.0 * MAX_POSITION))
    total = 0.0
    for c in range(N_CORES):
        outq = results[c]["outq"].astype(np.float64)  # [A, 2*NQ]
        outr = results[c]["outr"].astype(np.float64)  # [128, 4]
        qs = outq[:, :NQ].T.reshape(SS)               # s_local = 4g + j
        ql = outq[:, NQ:].T.reshape(SS)
        R = outr[:, 0:2].T.reshape(SS)                # s_local = 128b + p
        rlast = outr[:, 2:4].T.reshape(SS)
        L = -0.5 * qs / SD_VAR + T * C0
        ll_last = -0.5 * ql / SD_VAR + C0
        A_sum = R + rlast - ALPHA * (L + ll_last) - T * mx_pos
        total += np.sum(A_sum * L)
    out = np.float32(total / S)
    if _run_kwargs:
        _NC_CACHE["last_result"] = res
    return out


if __name__ == "__main__":
    rng = np.random.default_rng(0)
    inputs = {
        "states": rng.standard_normal((S, D, T), dtype=np.float32),
        "actions": rng.standard_normal((S, A, T), dtype=np.float32),
        "rewards": rng.standard_normal((S, T), dtype=np.float32),
        "W1": (rng.standard_normal((D, HID)) / np.sqrt(D)).astype(np.float32),
        "b1": np.zeros(HID, np.float32),
        "W2": (rng.standard_normal((HID, A)) / np.sqrt(HID)).astype(np.float32),
        "b2": np.zeros(A, np.float32),
    }
    print("result:", kernel(**inputs))


# revision 17
# speedup vs baseline: 2.3725x; 1.0424x over previous
"""Trainium2 Bass kernel for nn_MEPG_Loss (MEPG policy-gradient loss).

Math (forward only; stop_gradient is identity):
    h   = tanh(states[s,:,t] @ W1 + b1)                  [S,T,H]
    mu  = h @ W2 + b2                                    [S,T,A]
    ll[s,t] = -0.5*(||a[s,:,t]-mu||^2/SD + A*log(2*pi*SD))
    base = rewards.T - ALPHA*ll.T ; cum = base with row T-2 += row T-1
    A_hat = cum - log(0.5)
    out = einsum('ts,us->', A_hat, ll.T)/S
        = sum_s (sum_t A_hat[t,s]) * (sum_t ll[t,s]) / S

So only per-simulation reductions are needed:
    q_sum[s]  = sum_{t,d} (mu - a)^2,   q_last[s] = sum_d (mu - a)^2 at t=T-1
    R[s] = sum_t rewards,               r_last[s] = rewards[s,T-1]
    L = -0.5*q_sum/SD + T*C0 ;          ll_last = -0.5*q_last/SD + C0
    A_sum = R + r_last - ALPHA*(L + ll_last) - T*log(0.5)
    out = sum_s A_sum*L / S

Device layout (per core, 256 sims, processed as 64 quads of 4 sims,
grouped in blocks of 4 quads for DMA batching):
    - states quad loaded (bf16) at SBUF partitions {32j..32j+16}, j = sim-in-quad
    - mm1 (bf16): 4 row-tiled matmuls (K=16) -> h_pre [128, 512] psum fp32
    - tanh on ScalarE over 4 banks in one op -> h SBUF [128, 2048] bf16
    - mm2 (bf16): 4 col-tiled matmuls lhsT=W2 -> mu at psum partitions {32j+d}
    - diff (fp32): 4 diag-tiled identity matmuls accumulate (b2 - a) onto mu
    - DVE: copy diff psum->sbuf, then scalar_tensor_tensor squares it with
      free-axis sum into a per-quad accumulator column
    - final K=128 matmul with a 4-block selection matrix sums over d
Final combine (tiny) is done on host in float64.
"""

import os
import sys

import numpy as np

if not any(os.path.isdir(os.path.join(p, "concourse")) for p in sys.path if p):
    sys.path.insert(0, "/opt/trn_rl_repo")

import ml_dtypes

import concourse.bacc as bacc
import concourse.tile as tile
from concourse import mybir
from concourse.bass_utils import run_bass_kernel_spmd

# Problem constants (hardcoded per contract)
S, D, A, T, HID = 2048, 16, 4, 512, 128
N_CORES = 8
SS = S // N_CORES          # 256 sims per core
NQ = SS // 4               # 64 quads per core
QB = 8                     # quads per DMA block
NB = NQ // QB              # 16 blocks
SD_VAR = 0.04
ALPHA = 0.1
MAX_POSITION = 1.0

F32 = mybir.dt.float32
BF16 = mybir.dt.bfloat16
NP_BF16 = ml_dtypes.bfloat16


def _build_program():
    nc = bacc.Bacc("TRN2", target_bir_lowering=False, debug=False)

    states_d = nc.dram_tensor("states", [SS, D, T], BF16, kind="ExternalInput").ap()
    aadj_d = nc.dram_tensor("aadj", [SS, A, T], F32, kind="ExternalInput").ap()
    rew_d = nc.dram_tensor("rewards", [SS, T], F32, kind="ExternalInput").ap()
    w1f_d = nc.dram_tensor("w1full", [128, HID], BF16, kind="ExternalInput").ap()
    w2_d = nc.dram_tensor("w2", [HID, A], BF16, kind="ExternalInput").ap()
    sel_d = nc.dram_tensor("sel", [128, A], F32, kind="ExternalInput").ap()
    b1_d = nc.dram_tensor("b1col", [HID, 1], F32, kind="ExternalInput").ap()

    outq_d = nc.dram_tensor("outq", [A, 2 * NQ], F32, kind="ExternalOutput").ap()
    outr_d = nc.dram_tensor("outr", [128, 4], F32, kind="ExternalOutput").ap()

    with tile.TileContext(nc) as tc:
        with (
            tc.tile_pool(name="consts", bufs=1) as consts,
            tc.tile_pool(name="stp", bufs=2) as stp,
            tc.tile_pool(name="atp", bufs=1) as atp,
            tc.tile_pool(name="hsb", bufs=2) as hsb,
            tc.tile_pool(name="sdp", bufs=2) as sdp,
            tc.tile_pool(name="acc", bufs=1) as accp,
            tc.tile_pool(name="outs", bufs=1) as outp,
        ):
            # constants
            w1t = consts.tile([128, HID], BF16, tag="w1t")
            w2t = consts.tile([HID, A], BF16, tag="w2t")
            selt = consts.tile([128, A], F32, tag="selt")
            b1t = consts.tile([HID, 1], F32, tag="b1t")
            nc.sync.dma_start(out=w1t[:], in_=w1f_d)
            nc.sync.dma_start(out=w2t[:], in_=w2_d)
            nc.sync.dma_start(out=selt[:], in_=sel_d)
            nc.sync.dma_start(out=b1t[:], in_=b1_d)

            # per-quad accumulators (written one column per quad)
            accq = accp.tile([128, NQ], F32, tag="accq")
            qlg = accp.tile([128, NQ], F32, tag="qlg")

            psh_cm = tc.tile_pool(name="psh", bufs=1, space="PSUM")
            psm_cm = tc.tile_pool(name="psm", bufs=1, space="PSUM")
            psh = psh_cm.__enter__()
            psm = psm_cm.__enter__()
            # persistent mu psum banks; partitions outside {32j+d, d<4} must be
            # exactly zero (the DVE square reads the full tile), and the
            # matmuls below never write them, so zero once here.
            mu_tiles = [psm.tile([128, T], F32, tag=f"mu{i}", name=f"mu{i}")
                        for i in range(2)]
            for mt in mu_tiles:
                nc.vector.memset(mt[:], 0.0)
            at_tiles = [atp.tile([128, QB * T], F32, tag=f"at{i}", name=f"at{i}")
                        for i in range(2)]
            for att in at_tiles:
                nc.vector.memset(att[:], 0.0)

            for b in range(NB):
                s0 = 4 * QB * b
                # batched loads: one DMA per sim-slot j covers all QB quads
                st = stp.tile([128, QB * T], BF16, tag="st")
                at = at_tiles[b % 2]
                for j in range(4):
                    src = states_d[s0 + j:s0 + 4 * QB:4]          # [QB, D, T]
                    nc.sync.dma_start(
                        out=st[32 * j:32 * j + D, :].rearrange(
                            "d (q t) -> d q t", q=QB),
                        in_=src.rearrange("q d t -> d q t"),
                    )
                    asrc = aadj_d[s0 + j:s0 + 4 * QB:4]           # [QB, A, T]
                    nc.sync.dma_start(
                        out=at[32 * j:32 * j + A, :].rearrange(
                            "d (q t) -> d q t", q=QB),
                        in_=asrc.rearrange("q d t -> d q t"),
                    )

                for q in range(QB):
                    g = QB * b + q
                    # hpsA double-buffered: mm1 of quad g+1 fills the other
                    # hpsA slot while tanh of quad g is still reading; hpsB's
                    # refill is hidden under the next quad's tanhA
                    hpsA = psh.tile([128, 2 * T], F32, tag=f"hpsA{g % 2}",
                                    name=f"hpsA_{g}")
                    hpsB = psh.tile([128, 2 * T], F32, tag="hpsB")
                    for j in range(4):
                        dst = hpsA if j < 2 else hpsB
                        nc.tensor.matmul(
                            out=dst[:, T * (j % 2):T * (j % 2 + 1)],
                            lhsT=w1t[32 * j:32 * j + D, :],
                            rhs=st[32 * j:32 * j + D, T * q:T * (q + 1)],
                            start=True, stop=True,
                            tile_position=(32 * j, 0),
                        )

                    h = hsb.tile([128, 4 * T], BF16, tag="h")
                    nc.scalar.activation(
                        out=h[:, 0:2 * T], in_=hpsA[:],
                        func=mybir.ActivationFunctionType.Tanh,
                        bias=b1t[:], scale=1.0,
                    )
                    nc.scalar.activation(
                        out=h[:, 2 * T:4 * T], in_=hpsB[:],
                        func=mybir.ActivationFunctionType.Tanh,
                        bias=b1t[:], scale=1.0,
                    )

                    mu = mu_tiles[g % 2]
                    for j in range(4):
                        nc.tensor.matmul(
                            out=mu[32 * j:32 * j + A, :],
                            lhsT=w2t[:],
                            rhs=h[:, T * j:T * (j + 1)],
                            start=True, stop=True,
                            tile_position=(0, 32 * j),
                            skip_group_check=True,
                        )

                    # diff = mu + (b2 - a)
                    dfc = sdp.tile([128, T], F32, tag="dfc")
                    nc.vector.tensor_tensor(
                        out=dfc[:], in0=at[:, T * q:T * (q + 1)], in1=mu[:],
                        op=mybir.AluOpType.add,
                    )
                    sd = sdp.tile([128, T], F32, tag="sd")
                    nc.vector.scalar_tensor_tensor(
                        out=sd[:], in0=dfc[:], scalar=1.0, in1=dfc[:],
                        op0=mybir.AluOpType.mult, op1=mybir.AluOpType.mult,
                        accum_out=accq[:, g:g + 1],
                    )
                    nc.vector.tensor_copy(qlg[:, g:g + 1], sd[:, T - 1:T])

            # rewards: R and r_last for two blocks of 128 sims
            outr_sb = outp.tile([128, 4], F32, tag="outr")
            for rb in range(2):
                rw = stp.tile([128, T], F32, tag="rw")
                nc.sync.dma_start(out=rw[:], in_=rew_d[128 * rb:128 * rb + 128, :])
                nc.vector.tensor_reduce(
                    out=outr_sb[:, rb:rb + 1], in_=rw[:],
                    axis=mybir.AxisListType.X, op=mybir.AluOpType.add,
                )
                nc.vector.tensor_copy(outr_sb[:, 2 + rb:3 + rb], rw[:, T - 1:T])

            psm_cm.__exit__(None, None, None)
            psh_cm.__exit__(None, None, None)

            # sum over d: QS[j, g] = sum_d accq[32j+d, g]
            psq_cm = tc.tile_pool(name="psq", bufs=1, space="PSUM")
            psq = psq_cm.__enter__()
            outq_sb = outp.tile([A, 2 * NQ], F32, tag="outq")
            qs_ps = psq.tile([A, NQ], F32, tag="qs")
            ql_ps = psq.tile([A, NQ], F32, tag="ql")
            nc.tensor.matmul(out=qs_ps[:], lhsT=selt[:], rhs=accq[:],
                             start=True, stop=True)
            nc.tensor.matmul(out=ql_ps[:], lhsT=selt[:], rhs=qlg[:],
                             start=True, stop=True)
            nc.vector.tensor_copy(outq_sb[:, 0:NQ], qs_ps[:])
            nc.vector.tensor_copy(outq_sb[:, NQ:2 * NQ], ql_ps[:])

            nc.sync.dma_start(out=outq_d, in_=outq_sb[:])
            nc.sync.dma_start(out=outr_d, in_=outr_sb[:])
            psq_cm.__exit__(None, None, None)

    nc.finalize()
    return nc


_NC_CACHE = {}


def _get_program():
    if "nc" not in _NC_CACHE:
        _NC_CACHE["nc"] = _build_program()
    return _NC_CACHE["nc"]


def _make_consts(W1, b1, W2, b2):
    w1full = np.zeros((128, HID), dtype=NP_BF16)
    sel = np.zeros((128, A), dtype=np.float32)
    for j in range(4):
        w1full[32 * j:32 * j + D, :] = W1.astype(NP_BF16)
        for d in range(A):
            sel[32 * j + d, j] = 1.0
    return {
        "w1full": w1full,
        "w2": np.ascontiguousarray(W2.astype(NP_BF16)),
        "sel": sel,
        "b1col": np.ascontiguousarray(b1.astype(np.float32).reshape(HID, 1)),
    }


def kernel(states, actions, rewards, W1, b1, W2, b2, _run_kwargs=None):
    states = np.ascontiguousarray(np.asarray(states, dtype=np.float32)
                                  .astype(NP_BF16))
    actions = np.asarray(actions, dtype=np.float32)
    rewards = np.ascontiguousarray(np.asarray(rewards, dtype=np.float32))
    W1 = np.asarray(W1, dtype=np.float32)
    b1 = np.asarray(b1, dtype=np.float32)
    W2 = np.asarray(W2, dtype=np.float32)
    b2 = np.asarray(b2, dtype=np.float32)

    aadj = np.ascontiguousarray(b2[None, :, None] - actions)
    consts = _make_consts(W1, b1, W2, b2)

    in_maps = []
    for c in range(N_CORES):
        sl = slice(SS * c, SS * (c + 1))
        m = {
            "states": states[sl],
            "aadj": aadj[sl],
            "rewards": rewards[sl],
        }
        m.update(consts)
        in_maps.append(m)

    nc = _get_program()
    res = run_bass_kernel_spmd(nc, in_maps, core_ids=list(range(N_CORES)),
                               **(_run_kwargs or {}))
    results = res.results

    # host combine in float64
    C0 = -0.5 * A * np.log(2.0 * np.pi * SD_VAR)
    mx_pos = np.log(1.0 / (2.0 * MAX_POSITION))
    total = 0.0
    for c in range(N_CORES):
        outq = results[c]["outq"].astype(np.float64)  # [A, 2*NQ]
        outr = results[c]["outr"].astype(np.float64)  # [128, 4]
        qs = outq[:, :NQ].T.reshape(SS)               # s_local = 4g + j
        ql = outq[:, NQ:].T.reshape(SS)
        R = outr[:, 0:2].T.reshape(SS)                # s_local = 128b + p
        rlast = outr[:, 2:4].T.reshape(SS)
        L = -0.5 * qs / SD_VAR + T * C0
        ll_last = -0.5 * ql / SD_VAR + C0
        A_sum = R + rlast - ALPHA * (L + ll_last) - T * mx_pos
        total += np.sum(A_sum * L)
    out = np.float32(total / S)
    if _run_kwargs:
        _NC_CACHE["last_result"] = res
    return out


if __name__ == "__main__":
    rng = np.random.default_rng(0)
    inputs = {
        "states": rng.standard_normal((S, D, T), dtype=np.float32),
        "actions": rng.standard_normal((S, A, T), dtype=np.float32),
        "rewards": rng.standard_normal((S, T), dtype=np.float32),
        "W1": (rng.standard_normal((D, HID)) / np.sqrt(D)).astype(np.float32),
        "b1": np.zeros(HID, np.float32),
        "W2": (rng.standard_normal((HID, A)) / np.sqrt(HID)).astype(np.float32),
        "b2": np.zeros(A, np.float32),
    }
    print("result:", kernel(**inputs))


# revision 18
# speedup vs baseline: 2.8447x; 1.1990x over previous
"""Trainium2 Bass kernel for nn_MEPG_Loss (MEPG policy-gradient loss).

Math (forward only; stop_gradient is identity):
    h   = tanh(states[s,:,t] @ W1 + b1)                  [S,T,H]
    mu  = h @ W2 + b2                                    [S,T,A]
    ll[s,t] = -0.5*(||a[s,:,t]-mu||^2/SD + A*log(2*pi*SD))
    base = rewards.T - ALPHA*ll.T ; cum = base with row T-2 += row T-1
    A_hat = cum - log(0.5)
    out = einsum('ts,us->', A_hat, ll.T)/S
        = sum_s (sum_t A_hat[t,s]) * (sum_t ll[t,s]) / S

So only per-simulation reductions are needed:
    q_sum[s]  = sum_{t,d} (mu - a)^2,   q_last[s] = sum_d (mu - a)^2 at t=T-1
    R[s] = sum_t rewards,               r_last[s] = rewards[s,T-1]
    L = -0.5*q_sum/SD + T*C0 ;          ll_last = -0.5*q_last/SD + C0
    A_sum = R + r_last - ALPHA*(L + ll_last) - T*log(0.5)
    out = sum_s A_sum*L / S

Device layout (per core, 256 sims, processed as 64 quads of 4 sims,
grouped in blocks of 4 quads for DMA batching):
    - states quad loaded (bf16) at SBUF partitions {32j..32j+16}, j = sim-in-quad
    - mm1 (bf16): 4 row-tiled matmuls (K=16) -> h_pre [128, 512] psum fp32
    - tanh on ScalarE over 4 banks in one op -> h SBUF [128, 2048] bf16
    - mm2 (bf16): 4 col-tiled matmuls lhsT=W2 -> mu at psum partitions {32j+d}
    - diff (fp32): 4 diag-tiled identity matmuls accumulate (b2 - a) onto mu
    - DVE: copy diff psum->sbuf, then scalar_tensor_tensor squares it with
      free-axis sum into a per-quad accumulator column
    - final K=128 matmul with a 4-block selection matrix sums over d
Final combine (tiny) is done on host in float64.
"""

import os
import sys

import numpy as np

if not any(os.path.isdir(os.path.join(p, "concourse")) for p in sys.path if p):
    sys.path.insert(0, "/opt/trn_rl_repo")

import ml_dtypes

import concourse.bacc as bacc
import concourse.tile as tile
from concourse import mybir
from concourse.bass_utils import run_bass_kernel_spmd

# Problem constants (hardcoded per contract)
S, D, A, T, HID = 2048, 16, 4, 512, 128
N_CORES = 8
SS = S // N_CORES          # 256 sims per core
NQ = SS // 4               # 64 quads per core
QB = 8                     # quads per DMA block
NB = NQ // QB              # 16 blocks
SD_VAR = 0.04
ALPHA = 0.1
MAX_POSITION = 1.0

F32 = mybir.dt.float32
BF16 = mybir.dt.bfloat16
NP_BF16 = ml_dtypes.bfloat16


def _build_program():
    nc = bacc.Bacc("TRN2", target_bir_lowering=False, debug=False)

    states_d = nc.dram_tensor("states", [SS, D, T], BF16, kind="ExternalInput").ap()
    aadj_d = nc.dram_tensor("aadj", [SS, A, T], F32, kind="ExternalInput").ap()
    rew_d = nc.dram_tensor("rewards", [SS, T], F32, kind="ExternalInput").ap()
    w1f_d = nc.dram_tensor("w1full", [128, HID], BF16, kind="ExternalInput").ap()
    w2_d = nc.dram_tensor("w2", [HID, A], BF16, kind="ExternalInput").ap()
    sel_d = nc.dram_tensor("sel", [128, A], F32, kind="ExternalInput").ap()
    b1_d = nc.dram_tensor("b1col", [HID, 1], F32, kind="ExternalInput").ap()

    outq_d = nc.dram_tensor("outq", [A, 2 * NQ], F32, kind="ExternalOutput").ap()
    outr_d = nc.dram_tensor("outr", [128, 4], F32, kind="ExternalOutput").ap()

    with tile.TileContext(nc) as tc:
        with (
            tc.tile_pool(name="consts", bufs=1) as consts,
            tc.tile_pool(name="stp", bufs=2) as stp,
            tc.tile_pool(name="atp", bufs=1) as atp,
            tc.tile_pool(name="hsb", bufs=2) as hsb,
            tc.tile_pool(name="sdp", bufs=2) as sdp,
            tc.tile_pool(name="acc", bufs=1) as accp,
            tc.tile_pool(name="outs", bufs=1) as outp,
        ):
            # constants
            w1t = consts.tile([128, HID], BF16, tag="w1t")
            w2t = consts.tile([HID, A], BF16, tag="w2t")
            selt = consts.tile([128, A], F32, tag="selt")
            b1t = consts.tile([HID, 1], F32, tag="b1t")
            nc.sync.dma_start(out=w1t[:], in_=w1f_d)
            nc.sync.dma_start(out=w2t[:], in_=w2_d)
            nc.sync.dma_start(out=selt[:], in_=sel_d)
            nc.sync.dma_start(out=b1t[:], in_=b1_d)

            # per-quad accumulators (written one column per quad)
            accq = accp.tile([128, NQ], F32, tag="accq")
            qlg = accp.tile([128, NQ], F32, tag="qlg")

            psh_cm = tc.tile_pool(name="psh", bufs=1, space="PSUM")
            psm_cm = tc.tile_pool(name="psm", bufs=1, space="PSUM")
            psh = psh_cm.__enter__()
            psm = psm_cm.__enter__()
            # persistent mu psum banks; partitions outside {32j+d, d<4} must be
            # exactly zero (the DVE square reads the full tile), and the
            # matmuls below never write them, so zero once here.
            mu_tiles = [psm.tile([128, T], F32, tag=f"mu{i}", name=f"mu{i}")
                        for i in range(2)]
            for mt in mu_tiles:
                nc.vector.memset(mt[:], 0.0)
            at_tiles = [atp.tile([128, QB * T], F32, tag=f"at{i}", name=f"at{i}")
                        for i in range(2)]
            for att in at_tiles:
                nc.vector.memset(att[:], 0.0)

            at_of = {}

            def _tail_quad(g, h):
                at = at_tiles[(g // QB) % 2]
                q = g % QB
                mu = mu_tiles[g % 2]
                for j in range(4):
                    nc.tensor.matmul(
                        out=mu[32 * j:32 * j + A, :],
                        lhsT=w2t[:],
                        rhs=h[:, T * j:T * (j + 1)],
                        start=True, stop=True,
                        tile_position=(0, 32 * j),
                        skip_group_check=True,
                    )
                # diff = mu + (b2 - a)
                dfc = sdp.tile([128, T], F32, tag="dfc", name=f"dfc_{g}")
                nc.vector.tensor_tensor(
                    out=dfc[:], in0=at[:, T * q:T * (q + 1)], in1=mu[:],
                    op=mybir.AluOpType.add,
                )
                sd = sdp.tile([128, T], F32, tag="sd", name=f"sd_{g}")
                nc.vector.scalar_tensor_tensor(
                    out=sd[:], in0=dfc[:], scalar=1.0, in1=dfc[:],
                    op0=mybir.AluOpType.mult, op1=mybir.AluOpType.mult,
                    accum_out=accq[:, g:g + 1],
                )
                nc.vector.tensor_copy(qlg[:, g:g + 1], sd[:, T - 1:T])

            pipe = None
            for b in range(NB):
                s0 = 4 * QB * b
                # batched loads: one DMA per sim-slot j covers all QB quads
                st = stp.tile([128, QB * T], BF16, tag="st")
                at = at_tiles[b % 2]
                for j in range(4):
                    src = states_d[s0 + j:s0 + 4 * QB:4]          # [QB, D, T]
                    nc.sync.dma_start(
                        out=st[32 * j:32 * j + D, :].rearrange(
                            "d (q t) -> d q t", q=QB),
                        in_=src.rearrange("q d t -> d q t"),
                    )
                    asrc = aadj_d[s0 + j:s0 + 4 * QB:4]           # [QB, A, T]
                    nc.sync.dma_start(
                        out=at[32 * j:32 * j + A, :].rearrange(
                            "d (q t) -> d q t", q=QB),
                        in_=asrc.rearrange("q d t -> d q t"),
                    )

                for q in range(QB):
                    g = QB * b + q
                    # hpsA double-buffered: mm1 of quad g+1 fills the other
                    # hpsA slot while tanh of quad g is still reading; hpsB's
                    # refill is hidden under the next quad's tanhA
                    hpsA = psh.tile([128, 2 * T], F32, tag=f"hpsA{g % 2}",
                                    name=f"hpsA_{g}")
                    hpsB = psh.tile([128, 2 * T], F32, tag="hpsB")
                    for j in range(4):
                        dst = hpsA if j < 2 else hpsB
                        nc.tensor.matmul(
                            out=dst[:, T * (j % 2):T * (j % 2 + 1)],
                            lhsT=w1t[32 * j:32 * j + D, :],
                            rhs=st[32 * j:32 * j + D, T * q:T * (q + 1)],
                            start=True, stop=True,
                            tile_position=(32 * j, 0),
                        )

                    h = hsb.tile([128, 4 * T], BF16, tag="h", name=f"h_{g}")
                    nc.scalar.activation(
                        out=h[:, 0:2 * T], in_=hpsA[:],
                        func=mybir.ActivationFunctionType.Tanh,
                        bias=b1t[:], scale=1.0,
                    )
                    nc.scalar.activation(
                        out=h[:, 2 * T:4 * T], in_=hpsB[:],
                        func=mybir.ActivationFunctionType.Tanh,
                        bias=b1t[:], scale=1.0,
                    )

                    # software pipeline: tail work for the PREVIOUS quad, so
                    # the next quad's mm1 isn't queued behind mm2 on PE
                    if pipe is not None:
                        _tail_quad(*pipe)
                    pipe = (g, h)

            # rewards: R and r_last for two blocks of 128 sims
            outr_sb = outp.tile([128, 4], F32, tag="outr")
            for rb in range(2):
                rw = stp.tile([128, T], F32, tag="rw")
                nc.sync.dma_start(out=rw[:], in_=rew_d[128 * rb:128 * rb + 128, :])
                nc.vector.tensor_reduce(
                    out=outr_sb[:, rb:rb + 1], in_=rw[:],
                    axis=mybir.AxisListType.X, op=mybir.AluOpType.add,
                )
                nc.vector.tensor_copy(outr_sb[:, 2 + rb:3 + rb], rw[:, T - 1:T])

            if pipe is not None:
                _tail_quad(*pipe)

            psm_cm.__exit__(None, None, None)
            psh_cm.__exit__(None, None, None)

            # sum over d: QS[j, g] = sum_d accq[32j+d, g]
            psq_cm = tc.tile_pool(name="psq", bufs=1, space="PSUM")
            psq = psq_cm.__enter__()
            outq_sb = outp.tile([A, 2 * NQ], F32, tag="outq")
            qs_ps = psq.tile([A, NQ], F32, tag="qs")
            ql_ps = psq.tile([A, NQ], F32, tag="ql")
            nc.tensor.matmul(out=qs_ps[:], lhsT=selt[:], rhs=accq[:],
                             start=True, stop=True)
            nc.tensor.matmul(out=ql_ps[:], lhsT=selt[:], rhs=qlg[:],
                             start=True, stop=True)
            nc.vector.tensor_copy(outq_sb[:, 0:NQ], qs_ps[:])
            nc.vector.tensor_copy(outq_sb[:, NQ:2 * NQ], ql_ps[:])

            nc.sync.dma_start(out=outq_d, in_=outq_sb[:])
            nc.sync.dma_start(out=outr_d, in_=outr_sb[:])
            psq_cm.__exit__(None, None, None)

    nc.finalize()
    return nc


_NC_CACHE = {}


def _get_program():
    if "nc" not in _NC_CACHE:
        _NC_CACHE["nc"] = _build_program()
    return _NC_CACHE["nc"]


def _make_consts(W1, b1, W2, b2):
    w1full = np.zeros((128, HID), dtype=NP_BF16)
    sel = np.zeros((128, A), dtype=np.float32)
    for j in range(4):
        w1full[32 * j:32 * j + D, :] = W1.astype(NP_BF16)
        for d in range(A):
            sel[32 * j + d, j] = 1.0
    return {
        "w1full": w1full,
        "w2": np.ascontiguousarray(W2.astype(NP_BF16)),
        "sel": sel,
        "b1col": np.ascontiguousarray(b1.astype(np.float32).reshape(HID, 1)),
    }


def kernel(states, actions, rewards, W1, b1, W2, b2, _run_kwargs=None):
    states = np.ascontiguousarray(np.asarray(states, dtype=np.float32)
                                  .astype(NP_BF16))
    actions = np.asarray(actions, dtype=np.float32)
    rewards = np.ascontiguousarray(np.asarray(rewards, dtype=np.float32))
    W1 = np.asarray(W1, dtype=np.float32)
    b1 = np.asarray(b1, dtype=np.float32)
    W2 = np.asarray(W2, dtype=np.float32)
    b2 = np.asarray(b2, dtype=np.float32)

    aadj = np.ascontiguousarray(b2[None, :, None] - actions)
    consts = _make_consts(W1, b1, W2, b2)

    in_maps = []
    for c in range(N_CORES):
        sl = slice(SS * c, SS * (c + 1))
        m = {
            "states": states[sl],
            "aadj": aadj[sl],
            "rewards": rewards[sl],
        }
        m.update(consts)
        in_maps.append(m)

    nc = _get_program()
    res = run_bass_kernel_spmd(nc, in_maps, core_ids=list(range(N_CORES)),
                               **(_run_kwargs or {}))
    results = res.results

    # host combine in float64
    C0 = -0.5 * A * np.log(2.0 * np.pi * SD_VAR)
    mx_pos = np.log(1.0 / (2.0 * MAX_POSITION))
    total = 0.0
    for c in range(N_CORES):
        outq = results[c]["outq"].astype(np.float64)  # [A, 2*NQ]
        outr = results[c]["outr"].astype(np.float64)  # [128, 4]
        qs = outq[:, :NQ].T.reshape(SS)               # s_local = 4g + j
        ql = outq[:, NQ:].T.reshape(SS)
        R = outr[:, 0:2].T.reshape(SS)                # s_local = 128b + p
        rlast = outr[:, 2:4].T.reshape(SS)
        L = -0.5 * qs / SD_VAR + T * C0
        ll_last = -0.5 * ql / SD_VAR + C0
        A_sum = R + rlast - ALPHA * (L + ll_last) - T * mx_pos
        total += np.sum(A_sum * L)
    out = np.float32(total / S)
    if _run_kwargs:
        _NC_CACHE["last_result"] = res
    return out


if __name__ == "__main__":
    rng = np.random.default_rng(0)
    inputs = {
        "states": rng.standard_normal((S, D, T), dtype=np.float32),
        "actions": rng.standard_normal((S, A, T), dtype=np.float32),
        "rewards": rng.standard_normal((S, T), dtype=np.float32),
        "W1": (rng.standard_normal((D, HID)) / np.sqrt(D)).astype(np.float32),
        "b1": np.zeros(HID, np.float32),
        "W2": (rng.standard_normal((HID, A)) / np.sqrt(HID)).astype(np.float32),
        "b2": np.zeros(A, np.float32),
    }
    print("result:", kernel(**inputs))
